# revision 18
# baseline (speedup 1.0000x reference)
"""Causal self-attention (B=8, T=1024, C=1024, H=16) on 8 TRN2 NeuronCores.

Sharding: pure data-parallel over batch — core b computes batch element b
with fully replicated weights (B == n_cores, so no collectives needed).

Per-core dataflow (bf16 matmuls, fp32 accumulation):
  1. x tiles DMA'd across the sync/scalar/gpsimd rings, cast to bf16
     (DVE / ScalarE / GpSimd round-robin), transposed on the PE in bf16
     (1 cycle/row instead of fp32's 2).
  2. Q^T/K^T emitted per head-pair tile [128, T] (even head rows 0..63,
     odd rows 64..127); weight blocks staged transiently, bqkv folded
     into the PSUM evacuation.
  3. V stored per (k-tile, pair) as two matmul lhsT blocks:
       lhsT0 = [v_even(64) | ones]            -> Y^T rows 0..63, sums row 64
       lhsT1 = [zeros(63) | ones | v_odd(64)] -> sums row 63, Y^T rows 64..127
     so the odd head's attention output lands directly at partitions
     63..127 (no partition-shift DMA), and each head's softmax
     denominators come for free from the ones column.
  4. Scores computed transposed S^T[k, q] per (head, k-tile) into
     1-bank PSUM tiles; causal structure skips k>q tiles; exp on
     ScalarE (scale=1/8 fused, no max subtraction needed — scores are
     O(1)); diagonal block masked multiplicatively after exp.
  5. The odd head's AV accumulates inside the k-loop; the even head's
     runs as a burst after it, so the pair's last-finishing head (even,
     rows 0..63) needs no shift. Normalization: gpsimd
     partition_broadcast of the sums row + DVE reciprocal + multiply —
     no DRAM roundtrips, no SBUF-to-SBUF shift DMAs.
  6. out = Y @ Wproj + bproj, evacuated with a broadcast bias add,
     output DMAs alternating across the sync/scalar rings.
"""

import numpy as np

import concourse.tile as tile
from concourse import bacc, mybir
from concourse.bass_utils import run_bass_kernel_spmd
from concourse.masks import make_identity

f32 = mybir.dt.float32
bf16 = mybir.dt.bfloat16
AF = mybir.ActivationFunctionType
ALU = mybir.AluOpType

B, T, C, H, HD = 8, 1024, 1024, 16, 64
P = 128
NT = T // P  # 8 token tiles
NS = C // P  # 8 contraction subtiles
NPAIR = H // 2  # 8 head pairs
# per-pair stride in v tiles; two 128-wide matmul lhsT blocks:
#   lhsT0 = [v0(64) | ones | zeros(63)]            -> y0 rows 0..63, sums0 row 64
#   lhsT1 = [zeros(32) | ones | zeros(31) | v1(64)] -> sums1 row 32, y1 rows 64..127
# (sums rows 64/32 keep all engine accesses 32-partition aligned)
VW = 256


def _build():
    nc = bacc.Bacc(trn_type="TRN2")
    x_d = nc.dram_tensor("x", (T, C), f32, kind="ExternalInput")
    wqkv_d = nc.dram_tensor("wqkv", (C, 3 * C), f32, kind="ExternalInput")
    bqkv_d = nc.dram_tensor("bqkv", (3 * C,), f32, kind="ExternalInput")
    wproj_d = nc.dram_tensor("wproj", (C, C), f32, kind="ExternalInput")
    bproj_d = nc.dram_tensor("bproj", (C,), f32, kind="ExternalInput")
    out_d = nc.dram_tensor("out", (T, C), f32, kind="ExternalOutput")

    with tile.TileContext(nc) as tc:
        with (
            tc.tile_pool(name="big", bufs=1) as big,
            tc.tile_pool(name="stage", bufs=4) as stage,
            tc.tile_pool(name="wqst", bufs=2) as wqst,
            tc.tile_pool(name="xbf", bufs=3) as xbfp,
            tc.tile_pool(name="wst", bufs=3) as wstp,
            tc.tile_pool(name="ptp", bufs=3) as ptp,
            tc.tile_pool(name="pt0p", bufs=1) as pt0p,
            tc.tile_pool(name="yup", bufs=3) as yup,
            tc.tile_pool(name="sbp", bufs=2) as sbp,
            tc.tile_pool(name="outp", bufs=3) as outp,
            tc.tile_pool(name="pmm", bufs=2, space="PSUM") as pmm,
            tc.tile_pool(name="punit", bufs=4, space="PSUM") as punit,
            tc.tile_pool(name="pyp", bufs=2, space="PSUM") as pyp,
        ):
            # ---------------- x + first weights: DMA spread over rings -----
            x_r = x_d[:, :].rearrange("(i p) c -> p i c", p=P)
            wq_r = wqkv_d[:, :].rearrange("(s p) i -> p s i", p=P)
            x_ring = [nc.sync, nc.scalar, nc.gpsimd]
            xsts = []
            for i in range(3):
                xst = stage.tile([P, C], f32, tag="stage", name=f"xst{i}")
                x_ring[i % 3].dma_start(xst, x_r[:, i, :])
                xsts.append(xst)

            # first Q/K weight m-blocks on dedicated staging (no pool wait)
            wq_first = []
            for idx, m in enumerate((0, C // P)):
                st = wqst.tile([P, C], f32, tag="wqst", name=f"wqk{m}")
                st3 = st.rearrange("p (s c) -> p s c", c=P)
                (nc.sync if idx == 0 else nc.scalar).dma_start(
                    st3, wq_r[:, :, m * P : (m + 1) * P]
                )
                wq_first.append(st3)

            for i in range(3, NT):
                xst = stage.tile([P, C], f32, tag="stage", name=f"xst{i}")
                x_ring[i % 3].dma_start(xst, x_r[:, i, :])
                xsts.append(xst)

            # constants (vector ring: only self-owned tiles, no pool waits)
            ident = big.tile([P, P], bf16, tag="ident")
            make_identity(nc, ident)
            cmask = big.tile([P, P], bf16, tag="cmask")
            nc.gpsimd.memset(cmask, 1.0)
            nc.gpsimd.affine_select(
                out=cmask,
                in_=cmask,
                compare_op=ALU.is_ge,
                fill=0.0,
                base=0,
                pattern=[[1, P]],
                channel_multiplier=-1,
            )
            ones_row = big.tile([P, HD], bf16, tag="ones_row")
            nc.gpsimd.memset(ones_row, 1.0)
            bqk_col = big.tile([P, 2 * C // P], f32, tag="bqk")
            nc.gpsimd.dma_start(bqk_col, bqkv_d[: 2 * C].rearrange("(o p) -> p o", p=P))
            bias_v = big.tile([P, C], f32, tag="bias_v")
            nc.gpsimd.dma_start(bias_v, bqkv_d[2 * C :][None, :].to_broadcast((P, C)))

            # V weight columns: DMA posts only (copies emitted later so the
            # DVE queue isn't head-blocked waiting on these transfers)
            wqkv_sb = big.tile([P, NS, C], bf16, tag="wqkv")  # V columns only
            vw_ring = [nc.gpsimd, nc.gpsimd, nc.sync, nc.scalar,
                       nc.gpsimd, nc.gpsimd, nc.sync, nc.scalar]
            vw_sts = []
            for s in range(NS):
                st = stage.tile([P, C], f32, tag="stage", name=f"vw{s}")
                vw_ring[s].dma_start(st, wq_r[:, s, 2 * C : 3 * C])
                vw_sts.append(st)

            # ---------------- transpose x (bf16) ----------------
            xt_sb = big.tile([P, NS, T], bf16, tag="xt")
            for i in range(NT):
                xbf = xbfp.tile([P, C], bf16, tag="xbf", name=f"xbf{i}")
                if i % 2 == 0:
                    nc.vector.tensor_copy(xbf, xsts[i])
                else:
                    nc.scalar.activation(xbf, xsts[i], AF.Copy)
                for jh in range(2):
                    tp = pmm.tile([P, 512], bf16, tag="pmm", name=f"xtp{i}_{jh}")
                    for jj in range(4):
                        j = 4 * jh + jj
                        nc.tensor.transpose(
                            tp[:, jj * P : (jj + 1) * P],
                            xbf[:, j * P : (j + 1) * P],
                            ident,
                        )
                    nc.vector.tensor_copy(
                        xt_sb[:, 4 * jh : 4 * jh + 4, i * P : (i + 1) * P],
                        tp.rearrange("p (j t) -> p j t", t=P),
                    )

            # ---------------- Q/K^T emission ----------------
            qkt_sb = big.tile([P, 2 * C // P, T], bf16, tag="qkt")
            wproj_sb = big.tile([P, NS, C], bf16, tag="wproj")
            wp_r = wproj_d[:, :].rearrange("(s p) j -> p s j", p=P)

            def emit_qk_from(st3, m):
                wbf = wstp.tile([P, NS, P], bf16, tag="wst", name=f"wbf{m}")
                nc.vector.tensor_copy(wbf, st3)
                for ch in range(2):
                    ps = pmm.tile([P, 512], f32, tag="pmm", name=f"qk{m}_{ch}")
                    for s in range(NS):
                        nc.tensor.matmul(
                            ps,
                            wbf[:, s, :],
                            xt_sb[:, s, ch * 512 : (ch + 1) * 512],
                            start=(s == 0),
                            stop=(s == NS - 1),
                        )
                    nc.vector.tensor_scalar_add(
                        qkt_sb[:, m, ch * 512 : (ch + 1) * 512],
                        ps,
                        bqk_col[:, m : m + 1],
                    )

            def emit_qk(m, ring):
                st = wqst.tile([P, C], f32, tag="wqst", name=f"wqk{m}")
                st3 = st.rearrange("p (s c) -> p s c", c=P)
                ring.dma_start(st3, wq_r[:, :, m * P : (m + 1) * P])
                emit_qk_from(st3, m)

            emit_qk_from(wq_first[0], 0)
            emit_qk_from(wq_first[1], C // P)

            # V weight casts (emitted after the startup rush)
            for s in range(NS):
                if s % 2 == 0:
                    nc.scalar.activation(wqkv_sb[:, s, :], vw_sts[s], AF.Copy)
                else:
                    nc.vector.tensor_copy(wqkv_sb[:, s, :], vw_sts[s])

            # ---------------- V (natural layout, per-pair lhsT blocks) -----
            v_sb = [
                big.tile([P, NPAIR * VW], bf16, tag=f"v{i}", name=f"v{i}")
                for i in range(NT)
            ]

            def emit_v(i):
                v4 = v_sb[i].rearrange("p (pr w) -> p pr w", w=VW)
                nc.gpsimd.memset(v4[:, :, 64:65], 1.0)  # ones (even head sums)
                nc.gpsimd.memset(v4[:, :, 65:160], 0.0)  # zeros pad
                nc.gpsimd.memset(v4[:, :, 160:161], 1.0)  # ones (odd head sums)
                nc.gpsimd.memset(v4[:, :, 161:192], 0.0)  # zeros pad
                for ch in range(2):
                    ps = pmm.tile([P, 512], f32, tag="pmm", name=f"v{i}_{ch}")
                    for s in range(NS):
                        nc.tensor.matmul(
                            ps,
                            xt_sb[:, s, i * P : (i + 1) * P],
                            wqkv_sb[:, s, ch * 512 : (ch + 1) * 512],
                            start=(s == 0),
                            stop=(s == NS - 1),
                        )
                    ps4 = ps.rearrange("p (pr two d) -> p pr two d", two=2, d=HD)
                    bias4 = bias_v[:, ch * 512 : (ch + 1) * 512].rearrange(
                        "p (pr two d) -> p pr two d", two=2, d=HD
                    )
                    nc.vector.tensor_tensor(
                        v4[:, 4 * ch : 4 * ch + 4, 0:HD],
                        ps4[:, :, 0, :],
                        bias4[:, :, 0, :],
                        ALU.add,
                    )
                    nc.vector.tensor_tensor(
                        v4[:, 4 * ch : 4 * ch + 4, 192:256],
                        ps4[:, :, 1, :],
                        bias4[:, :, 1, :],
                        ALU.add,
                    )

            # ---------------- attention ----------------
            yt_sb = [
                big.tile([P, T], bf16, tag=f"yt{g}", name=f"yt{g}") for g in range(NT)
            ]

            def norm_head(yu, sums_row, g, odd):
                # yu: sums at partition sums_row (64 even / 32 odd); y at
                # partitions 64..127 (odd) or 0..63 (even). Broadcast the
                # sums row across 64 partitions via a ones-row PE matmul,
                # then reciprocal + multiply on the DVE.
                lo, hi = (HD, P) if odd else (0, HD)
                sb = sbp.tile([P, T], bf16, tag="sb", name=f"sb{g}_{int(odd)}")
                for half in range(2):
                    cols = slice(half * 512, (half + 1) * 512)
                    bc = punit.tile(
                        [P, 512], f32, tag="ps", name=f"bc{g}_{int(odd)}_{half}"
                    )
                    nc.tensor.matmul(
                        bc[lo:hi, :],
                        ones_row[sums_row : sums_row + 1, :],
                        yu[sums_row : sums_row + 1, cols],
                        start=True,
                        stop=True,
                    )
                    with nc.allow_low_precision("softmax recips in bf16 (tol 2e-2)"):
                        nc.vector.reciprocal(sb[lo:hi, cols], bc[lo:hi, :])
                ysrc = yu[HD:P, :] if odd else yu[0:HD, :]
                nc.vector.tensor_tensor(yt_sb[g][lo:hi, :], ysrc, sb[lo:hi, :], ALU.mult)

            for g in range(NT):
                h0, h1 = 2 * g, 2 * g + 1
                m = g
                if g > 0:
                    emit_qk(m, nc.sync)
                    emit_qk((C // P) + m, nc.scalar)
                if g == 1:
                    for s in range(NS):
                        st = wqst.tile([P, C], f32, tag="wqst", name=f"wpst{s}")
                        (nc.sync if s % 2 == 0 else nc.scalar).dma_start(
                            st, wp_r[:, s, :]
                        )
                        nc.vector.tensor_copy(wproj_sb[:, s, :], st)
                qt0 = qkt_sb[0:HD, m, :]
                kt0 = qkt_sb[0:HD, (C // P) + m, :]
                qt1 = qkt_sb[HD:P, m, :]
                kt1 = qkt_sb[HD:P, (C // P) + m, :]
                # odd-head AV accumulators (full 128 rows via padded lhsT)
                ypA1 = pyp.tile([P, 512], f32, tag="py", name=f"ypA{h1}")
                ypB1 = pyp.tile([P, 512], f32, tag="py", name=f"ypB{h1}")
                yu1 = yup.tile([P, T], bf16, tag="yu", name=f"yu{h1}")
                pt0s = []
                for kt in range(NT):
                    if g == 0:
                        emit_v(kt)
                    q0 = kt * P
                    v4 = v_sb[kt].rearrange("p (pr w) -> p pr w", w=VW)
                    lhsT1 = v4[:, g, 128:256]  # [128, 128]: zeros|ones|zeros|v1
                    # --- odd head: scores -> exp -> mask -> AV (immediate)
                    pt1 = ptp.tile([P, T - q0], bf16, tag="pt", name=f"pt1_{g}_{kt}")
                    if kt <= 3:
                        spA = punit.tile([P, 512 - q0], f32, tag="ps", name=f"sA1_{g}_{kt}")
                        spB = punit.tile([P, 512], f32, tag="ps", name=f"sB1_{g}_{kt}")
                        nc.tensor.matmul(
                            spA, kt1[:, q0 : q0 + P], qt1[:, q0:512],
                            start=True, stop=True,
                        )
                        nc.tensor.matmul(
                            spB, kt1[:, q0 : q0 + P], qt1[:, 512:T],
                            start=True, stop=True,
                        )
                        nc.scalar.activation(pt1[:, 0 : 512 - q0], spA, AF.Exp, scale=0.125)
                        nc.vector.tensor_tensor(
                            pt1[:, 0:P], pt1[:, 0:P], cmask, ALU.mult
                        )
                        nc.scalar.activation(pt1[:, 512 - q0 :], spB, AF.Exp, scale=0.125)
                        nc.tensor.matmul(
                            ypA1[:, q0:512], lhsT1, pt1[:, 0 : 512 - q0],
                            start=(kt == 0), stop=(kt == 3),
                        )
                        nc.tensor.matmul(
                            ypB1, lhsT1, pt1[:, 512 - q0 :],
                            start=(kt == 0), stop=(kt == NT - 1),
                        )
                    else:
                        spB = punit.tile([P, T - q0], f32, tag="ps", name=f"sB1_{g}_{kt}")
                        nc.tensor.matmul(
                            spB, kt1[:, q0 : q0 + P], qt1[:, q0:T],
                            start=True, stop=True,
                        )
                        nc.scalar.activation(pt1, spB, AF.Exp, scale=0.125)
                        nc.vector.tensor_tensor(
                            pt1[:, 0:P], pt1[:, 0:P], cmask, ALU.mult
                        )
                        nc.tensor.matmul(
                            ypB1[:, q0 - 512 : 512], lhsT1, pt1,
                            start=False, stop=(kt == NT - 1),
                        )
                    if kt == 3:
                        # ypA1 complete: evacuate to free its bank
                        nc.vector.tensor_copy(yu1[:, 0:512], ypA1)
                    # --- even head: scores -> exp -> mask (AV deferred)
                    pt0 = pt0p.tile([P, T - q0], bf16, tag=f"pt0_{kt}", name=f"pt0_{g}_{kt}")
                    if kt <= 3:
                        spA0 = punit.tile([P, 512 - q0], f32, tag="ps", name=f"sA0_{g}_{kt}")
                        spB0 = punit.tile([P, 512], f32, tag="ps", name=f"sB0_{g}_{kt}")
                        nc.tensor.matmul(
                            spA0, kt0[:, q0 : q0 + P], qt0[:, q0:512],
                            start=True, stop=True,
                        )
                        nc.tensor.matmul(
                            spB0, kt0[:, q0 : q0 + P], qt0[:, 512:T],
                            start=True, stop=True,
                        )
                        nc.scalar.activation(pt0[:, 0 : 512 - q0], spA0, AF.Exp, scale=0.125)
                        nc.scalar.activation(pt0[:, 512 - q0 :], spB0, AF.Exp, scale=0.125)
                    else:
                        spB0 = punit.tile([P, T - q0], f32, tag="ps", name=f"sB0_{g}_{kt}")
                        nc.tensor.matmul(
                            spB0, kt0[:, q0 : q0 + P], qt0[:, q0:T],
                            start=True, stop=True,
                        )
                        nc.scalar.activation(pt0, spB0, AF.Exp, scale=0.125)
                    nc.vector.tensor_tensor(pt0[:, 0:P], pt0[:, 0:P], cmask, ALU.mult)
                    pt0s.append(pt0)
                # finish odd head (sums at row 32, y at 64..127)
                nc.vector.tensor_copy(yu1[:, 512:T], ypB1)
                norm_head(yu1, 32, g, odd=True)
                # even head AV burst (A half then B half, 1 bank each)
                v4g = [
                    v_sb[kt].rearrange("p (pr w) -> p pr w", w=VW)[:, g, 0:128]
                    for kt in range(NT)
                ]
                yu0 = yup.tile([P, T], bf16, tag="yu", name=f"yu{h0}")
                ypA0 = punit.tile([P, 512], f32, tag="ps", name=f"yA{h0}")
                for kt in range(4):
                    q0 = kt * P
                    nc.tensor.matmul(
                        ypA0[:, q0:512], v4g[kt], pt0s[kt][:, 0 : 512 - q0],
                        start=(kt == 0), stop=(kt == 3),
                    )
                nc.vector.tensor_copy(yu0[:, 0:512], ypA0)
                ypB0 = punit.tile([P, 512], f32, tag="ps", name=f"yB{h0}")
                for kt in range(NT):
                    q0 = kt * P
                    if kt <= 3:
                        nc.tensor.matmul(
                            ypB0, v4g[kt], pt0s[kt][:, 512 - q0 :],
                            start=(kt == 0), stop=(kt == NT - 1),
                        )
                    else:
                        nc.tensor.matmul(
                            ypB0[:, q0 - 512 : 512], v4g[kt], pt0s[kt],
                            start=False, stop=(kt == NT - 1),
                        )
                nc.vector.tensor_copy(yu0[:, 512:T], ypB0)
                norm_head(yu0, HD, g, odd=False)
                if g == 6:
                    # proj bias load (reuses the V bias tile; V phase is done)
                    nc.gpsimd.dma_start(
                        bias_v, bproj_d[:][None, :].to_broadcast((P, C))
                    )

            # ---------------- output projection ----------------
            bias_o = bias_v
            out_r = out_d[:, :].rearrange("(i p) j -> p i j", p=P)
            for i in range(NT):
                for ch in range(2):
                    ps = pmm.tile([P, 512], f32, tag="pmm", name=f"proj{i}_{ch}")
                    for g in range(NT):
                        nc.tensor.matmul(
                            ps,
                            yt_sb[g][:, i * P : (i + 1) * P],
                            wproj_sb[:, g, ch * 512 : (ch + 1) * 512],
                            start=(g == 0),
                            stop=(g == NT - 1),
                        )
                    ot = outp.tile([P, 512], f32, tag="out")
                    nc.vector.tensor_tensor(
                        ot, ps, bias_o[:, ch * 512 : (ch + 1) * 512], ALU.add
                    )
                    (nc.sync if (2 * i + ch) % 2 == 0 else nc.scalar).dma_start(
                        out_r[:, i, ch * 512 : (ch + 1) * 512], ot
                    )

    nc.compile()
    return nc


_NC = None


def _get_nc():
    global _NC
    if _NC is None:
        _NC = _build()
    return _NC


def _in_maps(x, Wqkv, bqkv, Wproj, bproj):
    x = np.ascontiguousarray(np.asarray(x, dtype=np.float32))
    shared = {
        "wqkv": np.ascontiguousarray(np.asarray(Wqkv, dtype=np.float32)),
        "bqkv": np.ascontiguousarray(np.asarray(bqkv, dtype=np.float32)),
        "wproj": np.ascontiguousarray(np.asarray(Wproj, dtype=np.float32)),
        "bproj": np.ascontiguousarray(np.asarray(bproj, dtype=np.float32)),
    }
    return [{"x": np.ascontiguousarray(x[b]), **shared} for b in range(B)]


def run(x, Wqkv, bqkv, Wproj, bproj, **run_kwargs):
    """Run on 8 cores; returns (output [B,T,C] fp32, BassKernelResults)."""
    nc = _get_nc()
    res = run_bass_kernel_spmd(
        nc, _in_maps(x, Wqkv, bqkv, Wproj, bproj), core_ids=list(range(B)), **run_kwargs
    )
    out = np.stack([res.results[b]["out"] for b in range(B)]).astype(np.float32)
    return out, res


def kernel(x, Wqkv, bqkv, Wproj, bproj, n_head=None, **_ignored):
    out, _ = run(x, Wqkv, bqkv, Wproj, bproj)
    return out
